# revision 12
# baseline (speedup 1.0000x reference)
"""Bass/Trainium2 kernel for FLAOperator(mode='gla') CPU-fallback scan.

Reference recurrence (per b, h, d lane, over t = 0..N-1):
    s_t = s_{t-1} + sigmoid(q_t * k_t + g_t) * v_t ;  y_t = s_t
i.e. y = cumsum over N of u, with u = sigmoid(q*k + g) * v  (pure elementwise).

Shapes: q,k,v,g,y all [B=2, H=16, N=4096, D=128] f32.

Strategy (8 NeuronCores, SPMD, no collectives).  The kernel is HBM/DVE
bound; per core it processes 4 of the 32 independent (b,h) recurrences
as 8 units of 2048 time steps:

  - Host prep (layout only): per (b,h) slab transpose to [D, N], split
    into 2 time chunks, de-interleave each into even/odd halves.  q,k,g
    are packed into ONE fp8(E3M4) tensor laid out so each unit's load is
    a single DMA with one contiguous 6 KiB run per partition; v and y
    stay bf16 (their rounding feeds the output directly; fp8 on q,k,g
    costs ~0.6% relative error, gate is 2e-2).
  - Loads: qkg via SWDGE (gpsimd) cast-DMA fp8->bf16, so DVE ops run in
    the fast 2x packed-bf16 mode; v via HWDGE (sync).  One DMA each.
    Unit 0 instead loads raw fp8 via sync and runs 1x-mode DVE ops: the
    SWDGE path (Q7 descriptor generation) takes ~10us to warm up and the
    low-latency HWDGE path starts the pipeline much earlier.
  - Radix-2 scan: DVE tensor_tensor_scan has TWO data operands
    (state = (data0 op0 state) op1 data1), so scan(uE, uO, add, add)
    yields the odd-position cumsum in HALF the columns (the serial scan
    costs ~2 cycles/column; TT ops cost ~0.5).  Even positions are one
    subtract: yE = yO - uO.  Both run in place inside the u tile, and
    the store ships [yO | yE] as one DMA (host swaps them back).
  - Elementwise work per unit, all on whole [128, 2048] tiles: a = q*k
    (DVE), a += g (DVE), sigmoid (ACT), u = s*v (DVE in place).
  - Two-stage software pipeline (2-unit lookahead) so the ACT round-trip
    hides under the next units' DVE muls; chunk pairs chain the scan via
    initial = prev yO's last column.
"""

from contextlib import ExitStack

import ml_dtypes
import numpy as np

import concourse.bass as bass
import concourse.tile as tile
from concourse import bacc, mybir
from concourse.bass_utils import run_bass_kernel_spmd

B, H, N, D = 2, 16, 4096, 128
N_CORES = 8
BH = B * H                    # 32 independent recurrences
BH_PER_CORE = BH // N_CORES   # 4
P = 128                       # partitions (= D)
NCH = 2                       # time chunks per (b,h)
NT = N // NCH                 # time steps per chunk (2048)
N4 = NT // 2                  # columns per parity half (1024)
F32 = mybir.dt.float32
BF16 = mybir.dt.bfloat16
BF16_NP = ml_dtypes.bfloat16
F8E3 = mybir.dt.float8e3
F8E3_NP = ml_dtypes.float8_e3m4

RAW_UNITS = 1   # leading units on the raw-fp8/HWDGE ramp path
LOOKAHEAD = 2

_PROGRAM = None       # cached compiled Bass program (module-level)
LAST_RESULTS = None   # BassKernelResults of the last run (for test harness)


def _build_program() -> bass.Bass:
    nc = bacc.Bacc("TRN2", debug=False, num_devices=N_CORES)

    # qkg: [bh, chunk, d, tensor(q/k/g), parity, n] -> one unit load is a
    # single [128, 6144] tile with one 6 KiB contiguous run per partition.
    qkg_d = nc.dram_tensor(
        "qkg", [BH_PER_CORE, NCH, D, 3, 2, N4], F8E3, kind="ExternalInput"
    ).ap()
    v_d = nc.dram_tensor(
        "v", [BH_PER_CORE, NCH, D, 2, N4], BF16, kind="ExternalInput"
    ).ap()
    # y: [bh, chunk, d, half, n] where half 0 = odd-position results (yO),
    # half 1 = even-position results (yE) - matches the SBUF tile layout so
    # the store is one contiguous DMA; the host swaps the halves back.
    y_d = nc.dram_tensor(
        "y", [BH_PER_CORE, NCH, D, 2, N4], BF16, kind="ExternalOutput"
    ).ap()

    units = [(bh, c) for bh in range(BH_PER_CORE) for c in range(NCH)]
    NU = len(units)  # 8

    with tile.TileContext(nc) as tc, ExitStack() as ctx:
        const_pool = ctx.enter_context(tc.tile_pool(name="const", bufs=1))
        qkg_pool = ctx.enter_context(tc.tile_pool(name="qkg", bufs=NU))
        v_pool = ctx.enter_context(tc.tile_pool(name="v", bufs=NU))
        a_pool = ctx.enter_context(tc.tile_pool(name="a", bufs=5))

        # Dummy sigmoid so the ACT function table loads during the
        # framework preamble instead of stalling the first real unit.
        warm = const_pool.tile([P, 2], BF16, tag="warm")
        nc.vector.memset(warm[:], 0.0)
        nc.scalar.activation(warm[:], warm[:], mybir.ActivationFunctionType.Sigmoid)

        stage1 = {}   # unit -> (at, vt) awaiting stage 2
        prev_y = {}   # bh -> y tile of previous chunk (scan carry chain)

        def emit_stage1(u, raw=False):
            bh, c = u
            in_dt = F8E3 if raw else BF16
            eng = nc.sync if raw else nc.gpsimd
            xt = qkg_pool.tile([P, 3 * NT], in_dt, tag="qkg")
            vt = v_pool.tile([P, NT], BF16, tag="v")
            eng.dma_start(out=xt[:], in_=qkg_d[bh, c])
            nc.sync.dma_start(out=vt[:], in_=v_d[bh, c])
            qt, kt, gt = xt[:, :NT], xt[:, NT : 2 * NT], xt[:, 2 * NT :]
            at = a_pool.tile([P, NT], BF16, tag="a")
            nc.vector.tensor_mul(at[:], qt, kt)       # a = q*k
            nc.vector.tensor_add(at[:], at[:], gt)    # a += g
            nc.scalar.activation(
                at[:], at[:], mybir.ActivationFunctionType.Sigmoid
            )
            stage1[u] = (at, vt)

        def emit_stage2(u):
            bh, c = u
            at, vt = stage1.pop(u)
            nc.vector.tensor_mul(at[:], at[:], vt[:])  # u = s*v
            # yO = cumsum of (uE + uO) pairs: radix-2 scan over N4 columns,
            # written in place over uE (column t is read before written).
            init = prev_y[bh][:, N4 - 1 : N4] if c > 0 else 0.0
            nc.vector.tensor_tensor_scan(
                out=at[:, :N4], data0=at[:, :N4], data1=at[:, N4:],
                initial=init,
                op0=mybir.AluOpType.add, op1=mybir.AluOpType.add,
            )
            # yE = yO - uO, in place over uO.
            nc.vector.tensor_sub(at[:, N4:], at[:, :N4], at[:, N4:])
            nc.scalar.dma_start(out=y_d[bh, c], in_=at[:])  # [yO | yE]
            prev_y[bh] = at

        for i, u in enumerate(units):
            emit_stage1(u, raw=(i < RAW_UNITS))
            if i >= LOOKAHEAD:
                emit_stage2(units[i - LOOKAHEAD])
        for u in units[-LOOKAHEAD:]:
            emit_stage2(u)

    nc.compile()  # bacc backend: wait legalization, reg alloc, nop fusion
    return nc


def kernel(q: np.ndarray, k: np.ndarray, v: np.ndarray, g: np.ndarray) -> np.ndarray:
    global _PROGRAM, LAST_RESULTS
    if _PROGRAM is None:
        _PROGRAM = _build_program()

    def prep(x):
        # [B,H,N,D] f32 -> [BH, NCH, 2, D, N4]: per (b,h), time-major per
        # d lane, chunked then de-interleaved into even/odd steps.
        x = np.asarray(x, dtype=np.float32).reshape(BH, NCH, N4, 2, D)
        return x.transpose(0, 1, 3, 4, 2)

    # qkg: stack -> [BH, NCH, D, 3(tensor), 2(parity), N4] fp8
    qkg = np.stack([prep(q), prep(k), prep(g)], axis=2)        # [BH,NCH,3,2,D,N4]
    qkg = np.ascontiguousarray(qkg.transpose(0, 1, 4, 2, 3, 5)).astype(F8E3_NP)
    vp = np.ascontiguousarray(prep(v).transpose(0, 1, 3, 2, 4)).astype(BF16_NP)

    in_maps = []
    for i in range(N_CORES):
        s = slice(i * BH_PER_CORE, (i + 1) * BH_PER_CORE)
        in_maps.append({"qkg": qkg[s], "v": vp[s]})

    LAST_RESULTS = run_bass_kernel_spmd(_PROGRAM, in_maps, core_ids=list(range(N_CORES)))
    y = np.concatenate([r["y"] for r in LAST_RESULTS.results], axis=0)
    # y: [BH, NCH, D, 2, N4] with half 0 = odd positions, half 1 = even.
    # -> [BH, NCH, N4, 2(par), D] with (even, odd) order -> [B, H, N, D]
    y = y.transpose(0, 1, 4, 3, 2)[:, :, :, ::-1, :]
    return np.ascontiguousarray(y).astype(np.float32).reshape(B, H, N, D)
